# revision 27
# baseline (speedup 1.0000x reference)
"""NTM cell kernel for Trainium2 (8 NeuronCores, batch-parallel).

Strategy (per core, 8 batches):
  - prev_memory slice is cast-loaded f32->bf16 into SBUF (row-major M16) and
    xbar-transposed on-chip into per-chunk transposed tiles (T16).
  - All O(N*D) reductions run on the tensor engine:
      * content dots + sum-of-squares streams over T16 / T16^2
      * read-vector contraction over M16
  - new_memory is never materialized; its dot/norm/read contributions are
    expanded algebraically in terms of streams over the ORIGINAL memory.
  - Addressing chains (softmax/gate/shift/sharpen) run on DVE/ACT in a
    [128 x 64] layout (n = p*64 + c).
  - Only one ACT table set is used (exp/ln); sqrt/sigmoid/tanh/softplus are
    rewritten via exp/ln so no table reloads occur.
"""

import sys

sys.path.insert(0, "/opt/trn_rl_repo")

import numpy as np

import concourse.bass as bass
import concourse.tile as tile
from concourse import mybir

F32 = mybir.dt.float32
BF16 = mybir.dt.bfloat16
AF = mybir.ActivationFunctionType
OP = mybir.AluOpType

B, N, D, C, IN, S = 64, 8192, 64, 256, 128, 3
NCORES = 8
BL = B // NCORES          # batches per core
P = 128                   # partitions
CH = N // P               # 64 chunks per batch (n = p*64 + c)
NPAIR = CH // 2           # 32 transposed tiles per batch
EPS = 1e-8

# whead column map
KR0, KR1 = 0, 64
BR, GR = 64, 65
SR0, SR1 = 66, 69
GAMR = 69
KW0, KW1 = 70, 134
BW, GW = 134, 135
SW0, SW1 = 136, 139
GAMW = 139
E0, E1 = 140, 204
A0, A1 = 204, 268
NHEAD = 268

# scalar table rows (S8 cols -> SC rows -> BC blocks of 8)
Q_BET_W, Q_G_W, Q_OMG_W, Q_SW0, Q_SW1, Q_SW2, Q_GAM_W, Q_NK2_W = range(8)
Q_BET_R, Q_G_R, Q_OMG_R, Q_SR0, Q_SR1, Q_SR2, Q_GAM_R, Q_NK2_R = range(8, 16)
Q_AKR, Q_AA = 16, 17
NQ = 18

# ---------------------------------------------------------------------------
# workaround: the deployed walrus accepts only ONE sem-wait per instruction.
# After TileContext exits, hoist extra waits onto injected single-wait nops
# (drains on the SP engine, ENGINE_NOPs elsewhere).
# ---------------------------------------------------------------------------
import concourse.tile as tile_mod


def _split_multi_waits(nc):
    for f in nc.m.functions:
        for b in f.blocks:
            insts = b.instructions
            i = 0
            while i < len(insts):
                ins = insts[i]
                si = getattr(ins, "sync_info", None)
                if si is None or len(si.on_wait) <= 1:
                    i += 1
                    continue
                waits = list(si.on_wait)
                ins.sync_info = mybir.SyncInfo(
                    on_wait=[waits[-1]], on_update=list(si.on_update)
                )
                eng = nc.engines[ins.engine]
                new_nops = []
                for w in waits[:-1]:
                    nop = eng.isa(
                        nc.isa.Opcode.NEURON_ISA_TPB_OPCODE_NOP, {}
                    ).ins
                    nop.sync_info = mybir.SyncInfo(on_wait=[w], on_update=[])
                    new_nops.append(nop)
                for nop in new_nops:
                    for bb2 in f.blocks:
                        try:
                            bb2.instructions.remove(nop)
                            break
                        except ValueError:
                            pass
                for k, nop in enumerate(new_nops):
                    insts.insert(i + k, nop)
                i += len(new_nops) + 1


_orig_exit = tile_mod.TileContext.__exit__


def _patched_exit(self, *a, **k):
    import os
    r = _orig_exit(self, *a, **k)
    if not os.environ.get("NTM_NO_WAITFIX"):
        _split_multi_waits(self.nc)
    return r


if not getattr(tile_mod.TileContext, "_waitfix_patched", False):
    tile_mod.TileContext.__exit__ = _patched_exit
    tile_mod.TileContext._waitfix_patched = True


# ---------------------------------------------------------------------------
# kernel body
# ---------------------------------------------------------------------------

BIGC_W = 512 + NQ * 128        # ident|ones|permu|permd (128 each) + sel rows
WPK_W = C + NHEAD + NHEAD + 2  # wc0|wh0|wh1|bc
WPK2_W = C + NHEAD             # wc1|bh(row0)


def _build_module():
    nc = bass.Bass(dynamic_dma_scratch_size=16384)

    mem = nc.dram_tensor("mem", [BL, N, D], F32, kind="ExternalInput")
    xrv = nc.dram_tensor("xrv", [BL, IN + D], F32, kind="ExternalInput")
    pw2 = nc.dram_tensor("pw2", [2, BL, N], F32, kind="ExternalInput")
    bigc = nc.dram_tensor("bigc", [128, BIGC_W], F32, kind="ExternalInput")
    wpk = nc.dram_tensor("wpk", [128, WPK_W], F32, kind="ExternalInput")
    wpk2 = nc.dram_tensor("wpk2", [64, WPK2_W], F32, kind="ExternalInput")
    out_d = nc.dram_tensor("out", [BL, C + D], F32, kind="ExternalOutput")

    with tile.TileContext(nc) as tc:
        _emit(nc, tc, mem, xrv, pw2, bigc, wpk, wpk2, out_d)
    return nc


def _emit(nc, tc, mem, xrv, pw2, bigc, wpk, wpk2, out_d):
    from contextlib import ExitStack

    ctx = ExitStack()
    big = ctx.enter_context(tc.tile_pool(name="big", bufs=1))
    cons = ctx.enter_context(tc.tile_pool(name="cons", bufs=1))
    work = ctx.enter_context(tc.tile_pool(name="work", bufs=1))
    t16p = ctx.enter_context(tc.tile_pool(name="t16p", bufs=3))
    qallp = ctx.enter_context(tc.tile_pool(name="qallp", bufs=1))
    t2p = ctx.enter_context(tc.tile_pool(name="t2p", bufs=3))
    ps_stream = ctx.enter_context(tc.tile_pool(name="ps_stream", bufs=2, space="PSUM"))
    ps_misc = ctx.enter_context(tc.tile_pool(name="ps_misc", bufs=3, space="PSUM"))
    ps_trp = ctx.enter_context(tc.tile_pool(name="ps_trp", bufs=2, space="PSUM"))
    ps_rvp = ctx.enter_context(tc.tile_pool(name="ps_rvp", bufs=1, space="PSUM"))

    # ---------------- big memory tiles: 4 paired cast-loads ---------------
    # (few DMAs ahead of the transposes keeps tile's 8 DMA sem lanes from
    # chaining the loads behind unrelated transposes)
    m16p = [big.tile([P, 2, CH, D], BF16, tag=f"m16p_{j}", name=f"m16p_{j}")
            for j in range(BL // 2)]
    for j in range(BL // 2):
        nc.gpsimd.dma_start(
            out=m16p[j],
            in_=mem[2 * j:2 * j + 2].rearrange("b (p c) d -> p b c d", p=128),
        )

    def M16(b):
        return m16p[b // 2][:, b % 2]

    # ---------------- packed constants / weights to SBUF ------------------
    bigc_sb = cons.tile([128, BIGC_W], F32, tag="bigc")
    nc.sync.dma_start(out=bigc_sb, in_=bigc[:])
    wpk_sb = cons.tile([128, WPK_W], F32, tag="wpk")
    nc.sync.dma_start(out=wpk_sb, in_=wpk[:])
    wpk2_sb = cons.tile([64, WPK2_W], F32, tag="wpk2")
    nc.sync.dma_start(out=wpk2_sb, in_=wpk2[:])
    xrv_sb = cons.tile([BL, IN + D], F32, tag="xrv")
    nc.scalar.dma_start(out=xrv_sb, in_=xrv[:])
    pw_sb = cons.tile([128, 2, BL, CH], F32, tag="pw_sb")
    nc.scalar.dma_start(
        out=pw_sb, in_=pw2.rearrange("w b (p c) -> p w b c", p=128))

    ident_sb = bigc_sb[:, 0:128]
    ones_sb = bigc_sb[:, 128:256]
    permu_sb = bigc_sb[:, 256:384]
    permd_sb = bigc_sb[:, 384:512]
    sel_sb = bigc_sb[0:32, 512:BIGC_W]
    wc0 = wpk_sb[:, 0:C]
    wh0 = wpk_sb[:, C:C + NHEAD]
    wh1 = wpk_sb[:, C + NHEAD:C + 2 * NHEAD]
    bc_sb = wpk_sb[:, C + 2 * NHEAD:C + 2 * NHEAD + 2]
    wc1 = wpk2_sb[:, 0:C]
    bh_sb = wpk2_sb[0:1, C:C + NHEAD]
    xt_in = xrv_sb[:, 0:IN]
    rv_sb = xrv_sb[:, IN:IN + D]
    pw_w = pw_sb[:, 0]
    pw_r = pw_sb[:, 1]

    # ---------------- controller: hT = relu(W_ctrl^T @ ctrl_in^T + b) -------
    ps_xt = ps_misc.tile([128, 144], F32, tag="pm")
    nc.tensor.transpose(ps_xt[:, 0:BL], xt_in, ident_sb[0:BL, 0:BL])
    xT = work.tile([128, BL], F32, tag="xT")
    nc.vector.tensor_copy(xT, ps_xt[:, 0:BL])
    ps_rt = ps_misc.tile([128, 144], F32, tag="pm")
    nc.tensor.transpose(ps_rt[0:D, 0:BL], rv_sb, ident_sb[0:BL, 0:BL])
    rvT = work.tile([64, BL], F32, tag="rvT")
    nc.vector.tensor_copy(rvT, ps_rt[0:D, 0:BL])

    hT_sb = []
    for j in range(2):
        ps_h = ps_misc.tile([128, 144], F32, tag="pm")
        nc.tensor.matmul(ps_h[:, 0:BL], wc0[:, j * 128:(j + 1) * 128], xT,
                         start=True, stop=False)
        nc.tensor.matmul(ps_h[:, 0:BL], wc1[:, j * 128:(j + 1) * 128], rvT,
                         start=False, stop=True)
        h_j = work.tile([128, BL], F32, tag=f"hT{j}")
        nc.scalar.activation(h_j, ps_h[:, 0:BL], AF.Relu,
                             bias=bc_sb[:, j:j + 1], scale=1.0)
        hT_sb.append(h_j)

    # ---------------- head params P = h @ Whead + bhead ----------------
    ps_p = ps_misc.tile([BL, 512], F32, tag="pm")
    nc.tensor.matmul(ps_p[:, 0:NHEAD], hT_sb[0], wh0, start=True, stop=False)
    nc.tensor.matmul(ps_p[:, 0:NHEAD], hT_sb[1], wh1, start=False, stop=False)
    nc.tensor.matmul(ps_p[:, 0:NHEAD], ones_sb[0:1, 0:BL], bh_sb,
                     start=False, stop=True)
    p_sb = work.tile([BL, NHEAD], F32, tag="p_sb")
    nc.vector.tensor_copy(p_sb, ps_p[:, 0:NHEAD])

    # ---------------- VA: per-batch d-vectors [BL, 8*64] ----------------
    # vec order: 0 k_w, 1 k_r, 2 e*k_r, 3 a, 4 a*e, 5 ones, 6 e, 7 e^2
    va = work.tile([BL, 512], F32, tag="va")
    nc.vector.tensor_copy(va[:, 0:64], p_sb[:, KW0:KW1])
    nc.vector.tensor_copy(va[:, 64:128], p_sb[:, KR0:KR1])

    def _sigmoid(dst, src):  # dst = 1/(1+exp(-src))
        nc.scalar.activation(dst, src, AF.Exp, scale=-1.0)
        nc.vector.tensor_scalar_add(dst, dst, 1.0)
        nc.vector.reciprocal(dst, dst)

    # e = sigmoid(P_e) -> va[:, 384:448]
    _sigmoid(va[:, 384:448], p_sb[:, E0:E1])
    # a = tanh(P_a) = 1 - 2/(exp(2x)+1) -> va[:, 192:256]
    nc.scalar.activation(va[:, 192:256], p_sb[:, A0:A1], AF.Exp, scale=2.0)
    nc.vector.tensor_scalar_add(va[:, 192:256], va[:, 192:256], 1.0)
    nc.vector.reciprocal(va[:, 192:256], va[:, 192:256])
    nc.vector.tensor_scalar(va[:, 192:256], va[:, 192:256], -2.0, 1.0,
                            op0=OP.mult, op1=OP.add)
    # e*k_r, a*e, ones, e^2
    nc.vector.tensor_mul(va[:, 128:192], va[:, 384:448], va[:, 64:128])
    nc.vector.tensor_mul(va[:, 256:320], va[:, 192:256], va[:, 384:448])
    nc.vector.memset(va[:, 320:384], 1.0)
    nc.vector.tensor_mul(va[:, 448:512], va[:, 384:448], va[:, 384:448])

    # ---------------- VTD: transposed vectors with zero-halves --------------
    # VTD[p, half, vec, b]; half 0: rows 0-63 hold vec, rows 64-127 zero.
    vtd = work.tile([128, 2, 8, BL], BF16, tag="vtd")
    nc.vector.memset(vtd, 0.0)
    vapad = work.tile([BL, 8, 128], F32, tag="vapad")
    nc.vector.memset(vapad, 0.0)
    for v in range(8):
        nc.vector.tensor_copy(vapad[:, v, 64:128], va[:, v * 64:(v + 1) * 64])
    ps_top = ps_misc.tile([128, 144], F32, tag="pm")
    ps_bot = ps_misc.tile([128, 144], F32, tag="pm")
    for v in range(8):
        nc.tensor.transpose(ps_top[0:64, v * BL:(v + 1) * BL],
                            va[:, v * 64:(v + 1) * 64],
                            ident_sb[0:BL, 0:BL])
        nc.tensor.transpose(ps_bot[:, v * BL:(v + 1) * BL],
                            vapad[:, v, :], ident_sb[0:BL, 0:BL])
    nc.vector.tensor_copy(
        vtd[0:64].rearrange("p h v b -> p (h v b)")[:, 0:64],
        ps_top[0:64, 0:64])
    nc.vector.tensor_copy(
        vtd[64:128].rearrange("p h v b -> p (h v b)")[:, 64:128],
        ps_bot[64:128, 0:64])
    # f32 copies of e^T and a^T for the read-vector assembly
    eT_sb = work.tile([64, BL], F32, tag="eT_sb")
    nc.vector.tensor_copy(eT_sb, ps_top[0:64, 6 * BL:7 * BL])
    aT_sb = work.tile([64, BL], F32, tag="aT_sb")
    nc.vector.tensor_copy(aT_sb, ps_top[0:64, 3 * BL:4 * BL])

    # ---------------- per-batch scalars S8 [BL, 32] ----------------
    s8 = work.tile([BL, 32], F32, tag="s8")
    nc.vector.memset(s8, 0.0)
    tmp64 = work.tile([BL, 64], F32, tag="tmp64")

    def _softplus(dst, src):  # ln(1 + exp(src))
        nc.scalar.activation(dst, src, AF.Exp)
        nc.vector.tensor_scalar_add(dst, dst, 1.0)
        nc.scalar.activation(dst, dst, AF.Ln)

    def _softmax3(dst, src):
        ex3 = work.tile([BL, 3], F32, tag="ex3")
        nc.scalar.activation(ex3, src, AF.Exp)
        sm = work.tile([BL, 1], F32, tag="sm3")
        nc.vector.reduce_sum(sm, ex3, axis=mybir.AxisListType.X)
        nc.vector.reciprocal(sm, sm)
        nc.vector.tensor_scalar(dst, ex3, sm, None, op0=OP.mult)

    _softplus(s8[:, Q_BET_W:Q_BET_W + 1], p_sb[:, BW:BW + 1])
    _sigmoid(s8[:, Q_G_W:Q_G_W + 1], p_sb[:, GW:GW + 1])
    nc.vector.tensor_scalar(s8[:, Q_OMG_W:Q_OMG_W + 1],
                            s8[:, Q_G_W:Q_G_W + 1], -1.0, 1.0,
                            op0=OP.mult, op1=OP.add)
    _softmax3(s8[:, Q_SW0:Q_SW0 + 3], p_sb[:, SW0:SW1])
    _softplus(s8[:, Q_GAM_W:Q_GAM_W + 1], p_sb[:, GAMW:GAMW + 1])
    nc.vector.tensor_scalar_add(s8[:, Q_GAM_W:Q_GAM_W + 1],
                                s8[:, Q_GAM_W:Q_GAM_W + 1], 1.0)
    nc.vector.tensor_mul(tmp64, va[:, 0:64], va[:, 0:64])
    nc.vector.reduce_sum(s8[:, Q_NK2_W:Q_NK2_W + 1], tmp64,
                         axis=mybir.AxisListType.X)

    _softplus(s8[:, Q_BET_R:Q_BET_R + 1], p_sb[:, BR:BR + 1])
    _sigmoid(s8[:, Q_G_R:Q_G_R + 1], p_sb[:, GR:GR + 1])
    nc.vector.tensor_scalar(s8[:, Q_OMG_R:Q_OMG_R + 1],
                            s8[:, Q_G_R:Q_G_R + 1], -1.0, 1.0,
                            op0=OP.mult, op1=OP.add)
    _softmax3(s8[:, Q_SR0:Q_SR0 + 3], p_sb[:, SR0:SR1])
    _softplus(s8[:, Q_GAM_R:Q_GAM_R + 1], p_sb[:, GAMR:GAMR + 1])
    nc.vector.tensor_scalar_add(s8[:, Q_GAM_R:Q_GAM_R + 1],
                                s8[:, Q_GAM_R:Q_GAM_R + 1], 1.0)
    nc.vector.tensor_mul(tmp64, va[:, 64:128], va[:, 64:128])
    nc.vector.reduce_sum(s8[:, Q_NK2_R:Q_NK2_R + 1], tmp64,
                         axis=mybir.AxisListType.X)

    nc.vector.tensor_mul(tmp64, va[:, 192:256], va[:, 64:128])
    nc.vector.reduce_sum(s8[:, Q_AKR:Q_AKR + 1], tmp64,
                         axis=mybir.AxisListType.X)
    nc.vector.tensor_mul(tmp64, va[:, 192:256], va[:, 192:256])
    nc.vector.reduce_sum(s8[:, Q_AA:Q_AA + 1], tmp64,
                         axis=mybir.AxisListType.X)

    # transpose S8 -> SC [32, BL] and broadcast -> BC [128, NQ*8]
    ps_sc = ps_misc.tile([128, 144], F32, tag="pm")
    nc.tensor.transpose(ps_sc[0:32, 0:BL], s8, ident_sb[0:BL, 0:BL])
    sc_sb = work.tile([32, BL], F32, tag="sc_sb")
    nc.vector.tensor_copy(sc_sb, ps_sc[0:32, 0:BL])
    ps_bc = ps_misc.tile([128, 144], F32, tag="pm")
    for q in range(NQ):
        nc.tensor.matmul(ps_bc[:, q * BL:(q + 1) * BL],
                         sel_sb[:, q * 128:(q + 1) * 128], sc_sb,
                         start=True, stop=True)
    bc_all = work.tile([128, NQ * BL], F32, tag="bc_all")
    nc.vector.tensor_copy(bc_all, ps_bc[:, 0:NQ * BL])

    def BC(q, b):
        return bc_all[:, q * BL + b:q * BL + b + 1]

    # ---------------- output staging ----------------
    out_sb = work.tile([BL, C + D], F32, tag="out_sb")
    ps_ho = ps_misc.tile([128, 144], F32, tag="pm")
    nc.tensor.transpose(ps_ho[0:BL, 0:128], hT_sb[0], ident_sb)
    nc.vector.tensor_copy(out_sb[:, 0:128], ps_ho[0:BL, 0:128])
    ps_ho2 = ps_misc.tile([128, 144], F32, tag="pm")
    nc.tensor.transpose(ps_ho2[0:BL, 0:128], hT_sb[1], ident_sb)
    nc.vector.tensor_copy(out_sb[:, 128:256], ps_ho2[0:BL, 0:128])

    r1_sb = work.tile([64, BL], F32, tag="r1_sb")
    r2_sb = work.tile([64, BL], F32, tag="r2_sb")
    swr_sb = work.tile([1, BL], F32, tag="swr_sb")

    # ---------------- helpers for grouped heavy phase ----------------
    # chain groups: (start, size, engine) -- staggered so the last (small)
    # groups start late and finish fast
    GROUPS = [(0, 4), (4, 4)]
    GENG = [nc.vector, nc.gpsimd]
    GIDX = {}
    for _gi, (_gs, _gz) in enumerate(GROUPS):
        for _o in range(_gz):
            GIDX[_gs + _o] = (_gi, _o)

    def scb4(q, gs, gsz):
        base = bc_all[:, q * BL + gs:q * BL + gs + gsz]
        return bass.AP(tensor=base.tensor, offset=base.offset,
                       ap=[base.ap[0], base.ap[1], [0, 32], [0, 2]])

    def scb3(q, gs, gsz):
        base = bc_all[:, q * BL + gs:q * BL + gs + gsz]
        return bass.AP(tensor=base.tensor, offset=base.offset,
                       ap=[base.ap[0], base.ap[1], [0, CH]])

    def bc3(t8):
        base = t8[:, :]
        return bass.AP(tensor=base.tensor, offset=base.offset,
                       ap=[base.ap[0], base.ap[1], [0, CH]])

    def c4(t):
        return t.rearrange("p b (u w) -> p b u w", w=2)

    def psum_colsum_bcast(cs8, gsz, eps=None, tag="tot"):
        # one matmul with a full ones stationary both sums over partitions
        # and broadcasts the per-batch total to every output partition
        ps_t = ps_misc.tile([128, 144], F32, tag="pm")
        nc.tensor.matmul(ps_t[:, 0:gsz], ones_sb, cs8, start=True, stop=True)
        rt = work.tile([128, gsz], F32, tag=tag + "_rt", name=tag + "_rt")
        if eps is not None:
            nc.vector.tensor_scalar_add(rt, ps_t[:, 0:gsz], eps)
            nc.vector.reciprocal(rt, rt)
        else:
            nc.vector.reciprocal(rt, ps_t[:, 0:gsz])
        return rt

    def w_chain_all(eng, gi, gsz, dk_v, ssm_v, pw_all, qo, gs, dst):
        # generator: yields after each emitted instruction. Transient tiles
        # share tags across groups (groups run back-to-back, WAR deps are
        # naturally satisfied).
        def ctile(tag):
            return work.tile([P, gsz, CH], F32, tag=tag, name=tag)

        bet, g_, omg, s0, s1, s2, gam, nk2 = (qo + i for i in range(8))
        v = ctile("wc_v")
        eng.tensor_mul(c4(v), ssm_v, scb4(nk2, gs, gsz)); yield
        nc.scalar.activation(v, v, AF.Ln); yield
        inv = ctile("wc_inv")
        nc.scalar.activation(inv, v, AF.Exp, scale=-0.5); yield
        bs1 = ctile("wc_bs1")
        eng.tensor_mul(c4(bs1), dk_v, scb4(bet, gs, gsz)); yield
        bsim = ctile("wc_bsim")
        eng.tensor_mul(bsim, bs1, inv); yield
        ex = ctile("wc_ex")
        nc.scalar.activation(ex, bsim, AF.Exp); yield
        cs = work.tile([128, gsz], F32, tag="wc_cs", name="wc_cs")
        nc.vector.reduce_sum(cs, ex, axis=mybir.AxisListType.X); yield
        rtot = psum_colsum_bcast(cs, gsz, tag="wc_t1"); yield
        gt = work.tile([128, gsz], F32, tag="wc_gt", name="wc_gt")
        eng.tensor_mul(gt, rtot, bc_all[:, g_ * BL + gs:g_ * BL + gs + gsz])
        yield
        t9 = ctile("wc_t9")
        eng.tensor_mul(t9, pw_all, scb3(omg, gs, gsz)); yield
        wg = ctile("wc_wg")
        eng.tensor_mul(wg, ex, bc3(gt)); yield
        eng.tensor_add(wg, wg, t9); yield
        ps_sh = ps_misc.tile([128, 144], F32, tag="pm")
        nc.tensor.matmul(ps_sh[:, 0:gsz], permu_sb, wg[:, :, 0],
                         start=True, stop=True); yield
        nc.tensor.matmul(ps_sh[:, gsz:2 * gsz], permd_sb, wg[:, :, CH - 1],
                         start=True, stop=True); yield
        wgp1 = ctile("wc_wgp1")
        eng.tensor_copy(wgp1[:, :, 0:CH - 1], wg[:, :, 1:CH]); yield
        nc.vector.tensor_copy(wgp1[:, :, CH - 1], ps_sh[:, 0:gsz]); yield
        wgm1 = ctile("wc_wgm1")
        eng.tensor_copy(wgm1[:, :, 1:CH], wg[:, :, 0:CH - 1]); yield
        nc.vector.tensor_copy(wgm1[:, :, 0], ps_sh[:, gsz:2 * gsz]); yield
        ws = ctile("wc_ws")
        eng.tensor_mul(ws, wgp1, scb3(s0, gs, gsz)); yield
        t10 = ctile("wc_t10")
        eng.tensor_mul(t10, wg, scb3(s1, gs, gsz)); yield
        eng.tensor_add(ws, ws, t10); yield
        eng.tensor_mul(t10, wgm1, scb3(s2, gs, gsz)); yield
        eng.tensor_add(ws, ws, t10); yield
        lg = ctile("wc_lg")
        nc.scalar.activation(lg, ws, AF.Ln); yield
        eng.tensor_mul(lg, lg, scb3(gam, gs, gsz)); yield
        wp = ctile("wc_wp")
        nc.scalar.activation(wp, lg, AF.Exp); yield
        cs2 = work.tile([128, gsz], F32, tag="wc_cs2", name="wc_cs2")
        nc.vector.reduce_sum(cs2, wp, axis=mybir.AxisListType.X); yield
        rt2 = psum_colsum_bcast(cs2, gsz, eps=EPS, tag="wc_t2"); yield
        eng.tensor_mul(dst, wp, bc3(rt2)); yield

    # ---------------- streams: all batches ----------------
    # per-pair pipeline: one cast-load (SWDGE) -> one xbar transpose on the
    # sync queue -> squares (ACT h0 / DVE or gpsimd h1) -> stream matmuls
    qgrp = [qallp.tile([P, gz, 512], F32, tag=f"qall{gi}", name=f"qall{gi}")
            for gi, (_, gz) in enumerate(GROUPS)]
    # pair 0 is transposed on the (otherwise idle) PE during the load phase;
    # pairs 1-3 go through the xbar, staged after the loads in model time.
    identb = work.tile([128, 128], BF16, tag="identb")
    nc.vector.tensor_copy(identb, ident_sb)
    TR_MS = [None, 0.040, 0.047, 0.054]
    for j in range(BL // 2):
        t16b2 = t16p.tile([P, 64, 128], BF16, tag="t16b2", name="t16b2")
        m16f2 = m16p[j].rearrange("p b c d -> p (b c d)")
        if j == 0:
            for k in range(64):
                ps_t = ps_trp.tile([128, 128], BF16, tag="ptr")
                nc.tensor.transpose(ps_t, m16f2[:, 128 * k:128 * (k + 1)],
                                    identb)
                ceng = nc.vector if k % 2 == 0 else nc.scalar
                if k % 2 == 0:
                    ceng.tensor_copy(t16b2[:, k, :], ps_t)
                else:
                    ceng.copy(t16b2[:, k, :], ps_t)
        else:
            with tc.tile_wait_until(TR_MS[j]):
                for q in range(4):
                    teng = nc.sync if q % 2 == 0 else nc.scalar
                    teng.dma_start_transpose(
                        t16b2[:, q * 16:(q + 1) * 16],
                        m16f2[:, q * 2048:(q + 1) * 2048],
                    )
        for i in range(2):
            b = 2 * j + i
            gidx, off = GIDX[b]
            pb = ps_stream.tile([128, 512], F32, tag="pb")
            rhs_m = vtd[:, :, 0:5, b].rearrange("p h v -> p v h")
            rhs_s = vtd[:, :, 5:8, b].rearrange("p h v -> p v h")
            for g in range(2):
                t2 = t2p.tile([P, 16, 128], BF16, tag="t2")
                sq_src = t16b2[:, i * 32 + g * 16:i * 32 + (g + 1) * 16]
                sq_src = sq_src.rearrange("p a q -> p (a q)")
                sq_dst = t2.rearrange("p a q -> p (a q)")
                if g == 0 and b < 4:
                    nc.scalar.activation(sq_dst, sq_src, AF.Square)
                elif g == 0:
                    nc.vector.tensor_mul(sq_dst, sq_src, sq_src)
                elif b < 4:
                    nc.vector.tensor_mul(sq_dst, sq_src, sq_src)
                else:
                    nc.gpsimd.tensor_mul(sq_dst, sq_src, sq_src)
                for cp in range(g * 16, (g + 1) * 16):
                    nc.tensor.matmul(pb[:, cp * 16:cp * 16 + 10],
                                     t16b2[:, i * 32 + cp], rhs_m,
                                     start=True, stop=True)
                for cp in range(g * 16, (g + 1) * 16):
                    nc.tensor.matmul(pb[:, cp * 16 + 10:cp * 16 + 16],
                                     t2[:, cp - g * 16], rhs_s,
                                     start=True, stop=True)
            if b < 4:
                nc.vector.tensor_copy(qgrp[gidx][:, off, :], pb)
            else:
                nc.scalar.copy(qgrp[gidx][:, off, :], pb)

    # ---------------- per-group chains, lockstep-interleaved --------------
    # Both groups' instruction streams are emitted alternately so the
    # in-order ACT/PE/DVE queues interleave the two chains and they run
    # concurrently (group A elementwise on DVE, group B on gpsimd).
    wrvs = [None, None]

    def group_prog(gi, gs, gsz, eng, qall):
        q4 = qall.rearrange("p b (cp j) -> p b cp j", j=16)

        def ctile(tag):
            return work.tile([P, gsz, CH], F32, tag=tag, name=tag)

        def QV(q):
            return q4[:, :, :, 2 * q:2 * q + 2]

        # write head
        w_w = work.tile([P, gsz, CH], F32, tag=f"w_w{gi}", name=f"w_w{gi}")
        yield from w_chain_all(eng, gi, gsz, QV(0), QV(5),
                               pw_w[:, gs:gs + gsz], 0, gs, w_w)

        # read-head inputs via algebra
        dots_r = ctile("dots_r")
        t_a = ctile("alg_t")
        eng.tensor_scalar(c4(t_a), QV(2), -1.0, None, op0=OP.mult); yield
        eng.tensor_add(t_a, t_a, scb3(Q_AKR, gs, gsz)); yield
        eng.tensor_mul(t_a, w_w, t_a); yield
        eng.tensor_add(c4(dots_r), c4(t_a), QV(1)); yield

        ss_r = ctile("ss_r")
        a1 = ctile("alg_a1")
        eng.tensor_sub(c4(a1), QV(3), QV(6)); yield
        a2 = ctile("alg_a2")
        eng.tensor_scalar(c4(a2), QV(4), -2.0, None, op0=OP.mult); yield
        eng.tensor_add(a2, a2, scb3(Q_AA, gs, gsz)); yield
        eng.tensor_add(c4(a2), c4(a2), QV(7)); yield
        h1 = ctile("alg_h1")
        eng.tensor_mul(h1, w_w, a2); yield
        t_b = ctile("alg_tb")
        eng.tensor_scalar(t_b, a1, 2.0, None, op0=OP.mult); yield
        eng.tensor_add(h1, h1, t_b); yield
        eng.tensor_mul(h1, w_w, h1); yield
        eng.tensor_add(c4(ss_r), c4(h1), QV(5)); yield

        w_r = work.tile([P, gsz, CH], F32, tag=f"w_r{gi}", name=f"w_r{gi}")
        yield from w_chain_all(eng, gi, gsz, c4(dots_r), c4(ss_r),
                               pw_r[:, gs:gs + gsz], 8, gs, w_r)

        # read-vector weight prep for this group
        wrw = ctile("wrw")
        eng.tensor_mul(wrw, w_r, w_w); yield
        swc = work.tile([128, gsz], F32, tag="swc", name="swc")
        nc.vector.reduce_sum(swc, wrw, axis=mybir.AxisListType.X); yield
        ps_sw = ps_misc.tile([128, 144], F32, tag="pm")
        nc.tensor.matmul(ps_sw[0:gsz, 0:1], swc, ones_sb[:, 0:1],
                         start=True, stop=True); yield
        swr_c = work.tile([gsz, 1], F32, tag="swr_c", name="swr_c")
        nc.scalar.copy(swr_c, ps_sw[0:gsz, 0:1]); yield
        ps_swt = ps_misc.tile([128, 144], F32, tag="pm")
        nc.tensor.transpose(ps_swt[0:1, 0:gsz], swr_c,
                            ident_sb[0:gsz, 0:gsz]); yield
        nc.scalar.copy(swr_sb[:, gs:gs + gsz], ps_swt[0:1, 0:gsz]); yield

        wrv = work.tile([P, gsz, CH, 2], BF16, tag=f"wrv{gi}",
                        name=f"wrv{gi}")
        eng.tensor_copy(wrv[:, :, :, 0], w_r); yield
        eng.tensor_copy(wrv[:, :, :, 1], wrw); yield
        wrvs[gi] = wrv

    for gi, (gs, gsz) in enumerate(GROUPS):
        for _ in group_prog(gi, gs, gsz, GENG[gi], qgrp[gi]):
            pass

    # ---------------- read-vector contractions (all batches) --------------
    for gi, (gs, gsz) in enumerate(GROUPS):
        wrv = wrvs[gi]
        for bb in range(gsz):
            b = gs + bb
            ps_rv = ps_rvp.tile([2, 64], F32, tag="ps_rv")
            for c in range(CH):
                nc.tensor.matmul(ps_rv, wrv[:, bb, c, :], M16(b)[:, c, :],
                                 start=(c == 0), stop=(c == CH - 1))
            rv2 = work.tile([2, 64], F32, tag="rv2", name="rv2")
            nc.scalar.copy(rv2, ps_rv)
            ps_rvt = ps_misc.tile([128, 144], F32, tag="pm")
            nc.tensor.transpose(ps_rvt[0:64, 0:2], rv2, ident_sb[0:2, 0:2])
            nc.scalar.copy(r1_sb[:, b:b + 1], ps_rvt[0:64, 0:1])
            nc.scalar.copy(r2_sb[:, b:b + 1], ps_rvt[0:64, 1:2])

    # ---------------- read-vector assembly (all batches) ----------------
    ps_swb = ps_misc.tile([128, 144], F32, tag="pm")
    nc.tensor.matmul(ps_swb[0:64, 0:BL], ones_sb[0:1, 0:64], swr_sb,
                     start=True, stop=True)
    rvt = work.tile([64, BL], F32, tag="rvt", name="rvt")
    nc.vector.tensor_mul(rvt, eT_sb, r2_sb)          # e * r2
    nc.vector.tensor_sub(rvt, r1_sb, rvt)            # r1 - e*r2
    m3 = work.tile([64, BL], F32, tag="m3", name="m3")
    nc.vector.tensor_copy(m3, ps_swb[0:64, 0:BL])
    nc.vector.tensor_mul(m3, aT_sb, m3)              # a * sum(wr*ww)
    nc.vector.tensor_add(rvt, rvt, m3)
    ps_rvo = ps_misc.tile([128, 144], F32, tag="pm")
    nc.tensor.transpose(ps_rvo[0:BL, 0:64], rvt, ident_sb[0:64, 0:64])
    nc.vector.tensor_copy(out_sb[:, C:C + D], ps_rvo[0:BL, 0:64])

    nc.sync.dma_start(out=out_d[:], in_=out_sb)
    ctx.close()


# ---------------------------------------------------------------------------
# host-side driver
# ---------------------------------------------------------------------------
_NC = None


def _get_module():
    global _NC
    if _NC is None:
        _NC = _build_module()
    return _NC


def build_inmaps(inputs):
    f = lambda k: np.ascontiguousarray(np.asarray(inputs[k], np.float32))

    whead = np.concatenate([
        f("Wk_r"), f("Wb_r"), f("Wg_r"), f("Ws_r"), f("Wgam_r"),
        f("Wk_w"), f("Wb_w"), f("Wg_w"), f("Ws_w"), f("Wgam_w"),
        f("We_w"), f("Wa_w")], axis=1)
    bhead = np.concatenate([
        f("bk_r"), f("bb_r"), f("bg_r"), f("bs_r"), f("bgam_r"),
        f("bk_w"), f("bb_w"), f("bg_w"), f("bs_w"), f("bgam_w"),
        f("be_w"), f("ba_w")])
    wctrl = f("W_ctrl")
    bctrl = f("b_ctrl")

    bigc = np.zeros((128, BIGC_W), np.float32)
    bigc[:, 0:128] = np.eye(128, dtype=np.float32)
    bigc[:, 128:256] = 1.0
    for m in range(128):
        bigc[(m + 1) % 128, 256 + m] = 1.0        # permu
        bigc[(m - 1) % 128, 384 + m] = 1.0        # permd
    for q in range(NQ):
        bigc[q, 512 + q * 128:512 + (q + 1) * 128] = 1.0  # sel rows

    wpk = np.zeros((128, WPK_W), np.float32)
    wpk[:, 0:C] = wctrl[0:128]
    wpk[:, C:C + NHEAD] = whead[0:128]
    wpk[:, C + NHEAD:C + 2 * NHEAD] = whead[128:256]
    wpk[:, C + 2 * NHEAD:C + 2 * NHEAD + 2] = bctrl.reshape(2, 128).T

    wpk2 = np.zeros((64, WPK2_W), np.float32)
    wpk2[:, 0:C] = wctrl[128:192]
    wpk2[0, C:C + NHEAD] = bhead

    mem = f("prev_memory")
    xrv = np.concatenate([f("x"), f("prev_read_vector")], axis=1)
    pw2 = np.stack([f("prev_write_weights"), f("prev_read_weights")])

    shared = dict(bigc=bigc, wpk=wpk, wpk2=wpk2)
    in_maps = []
    for c in range(NCORES):
        sl = slice(c * BL, (c + 1) * BL)
        in_maps.append(dict(
            mem=np.ascontiguousarray(mem[sl]),
            xrv=np.ascontiguousarray(xrv[sl]),
            pw2=np.ascontiguousarray(pw2[:, sl]),
            **shared))
    return in_maps


def kernel(**inputs):
    from concourse.bass_utils import run_bass_kernel_spmd

    nc = _get_module()
    in_maps = build_inmaps(inputs)
    res = run_bass_kernel_spmd(nc, in_maps, list(range(NCORES)))
    return np.concatenate([res.results[c]["out"] for c in range(NCORES)],
                          axis=0).astype(np.float32)

